# revision 10
# baseline (speedup 1.0000x reference)
"""MinLSTM cell for Trainium2 (Bass/Tile), data-parallel over batch on 8 cores.

Per core (one batch row), software-pipelined in two phases per time-chunk:
  A(ci): f,i projections (bf16 matmuls, K=768 in PSUM) + the whole gate
         chain through a=(1+Ei)/(2+Ef+Ei) and u=1-a (no h dependency).
  B(ci): h~ projection + b = (zh+bh)*u, the tensor_tensor_scan, and the
         output store.
Emission order A(0) A(1) B(0) A(2) B(1) ... A(n) B(n-1) B(n) keeps every
engine fed and leaves only the last chunk's h-matmuls + b/scan/store after
the final A — the last two chunks are 256 wide to shrink that tail.

DMA queues (each ~88 GB/s, 3 usable): sync=x chunks, gpsimd=wf + h-output
stores, scalar=wi, wh, biases. Chunk 0's x comes as 6 per-kd tiles so the
first matmul only waits for one kd slice; later chunks are one DMA each.

Gates are division-free via Exp/Ln from one LUT table: Ef=e^{-zf},
Ei=e^{-zi} (ScalarE Exp from PSUM, bias fused), 1/s2 = Exp(-Ln(ss+2)).
Engines: ACT {ef, ei, ln, rt}, Pool {ss, ut}, DVE {at, bt, scan}.
Output hT [H,T] bf16; host transposes/upcasts to [T,H] fp32.
"""

import sys

if "/opt/trn_rl_repo" not in sys.path:
    sys.path.insert(0, "/opt/trn_rl_repo")

import numpy as np

B, T, D, H = 8, 4096, 768, 768
TC = 512                    # steady-state time-chunk (one PSUM bank of fp32)
KD = D // 128               # 6 contraction tiles
MH = H // 128               # 6 hidden tiles
# (time offset, width, x-load index, offset within load)
CHUNKS = [(c * TC, TC, c, 0) for c in range(7)] + [(3584, 256, 7, 0), (3840, 256, 7, 256)]
XLOADS = [(c * TC, TC) for c in range(8)]

_state = {}


def _build():
    import concourse.mybir as mybir
    import concourse.tile as tile
    from concourse import bacc

    f32, bf16 = mybir.dt.float32, mybir.dt.bfloat16
    A = mybir.AluOpType
    Act = mybir.ActivationFunctionType

    nc = bacc.Bacc("TRN2", target_bir_lowering=False, debug=False, num_devices=B)

    xP = nc.dram_tensor("xP", [128, KD, T], bf16, kind="ExternalInput")
    w_d = {p: nc.dram_tensor(f"w{p}", [128, KD, H], bf16, kind="ExternalInput") for p in "fih"}
    bc_d = nc.dram_tensor("bcat", [128, 4 * MH], f32, kind="ExternalInput")
    hT = nc.dram_tensor("hT", [H, T], bf16, kind="ExternalOutput")

    with tile.TileContext(nc) as tc:
        with (
            tc.tile_pool(name="wpool", bufs=1) as wpool,
            tc.tile_pool(name="cpool", bufs=1) as cpool,
            tc.tile_pool(name="xpool", bufs=3) as xpool,
            tc.tile_pool(name="pspool", bufs=8, space="PSUM") as pspool,
            tc.tile_pool(name="wk", bufs=6) as wk,
            tc.tile_pool(name="hpool", bufs=3) as hpool,
        ):
            # Chunk-0 x as per-kd tiles on sync so the first matmul waits for
            # one kd slice only; weights as kd-pairs: wf on gpsimd, wi then
            # wh on scalar (wh isn't needed until B(0), ~20us in).
            x0k = []
            for kd in range(KD):
                t = xpool.tile([128, TC], bf16, tag=f"x0k{kd}", name=f"x0k{kd}")
                nc.sync.dma_start(t[:], xP[:, kd, 0:TC])
                x0k.append(t)
            w_sb = {p: [] for p in "fih"}
            for p, q in (("f", nc.gpsimd), ("i", nc.scalar), ("h", nc.scalar)):
                for kp in range(KD // 2):
                    t = wpool.tile([128, 2, H], bf16, tag=f"w{p}{kp}", name=f"w{p}{kp}")
                    q.dma_start(t[:], w_d[p][:, 2 * kp:2 * kp + 2, :])
                    w_sb[p].append(t)
            bcat = cpool.tile([128, 4 * MH], f32, tag="bc")
            nc.scalar.dma_start(bcat[:], bc_d[:])
            b_sb = {"f": bcat[:, 0:MH], "i": bcat[:, MH:2 * MH], "h": bcat[:, 2 * MH:3 * MH]}
            h0_sb = bcat[:, 3 * MH:4 * MH]
            two_sb = cpool.tile([128, 1], f32, tag="two")
            nc.gpsimd.memset(two_sb[:], 2.0)

            xtiles = [None] * len(XLOADS)

            def xslice(ci, kd):
                _, w, li, off = CHUNKS[ci]
                if li == 0:
                    return x0k[kd][:, 0:w]
                return xtiles[li][:, kd, off:off + w]

            def load_x(li):
                if li == 0 or xtiles[li] is not None:
                    return
                t0x, wx = XLOADS[li]
                xt = xpool.tile([128, KD, TC], bf16, tag="x", name=f"x{li}")
                nc.sync.dma_start(xt[:, :, 0:wx], xP[:, :, t0x:t0x + wx])
                xtiles[li] = xt

            PS = [[dict() for _ in range(MH)] for _ in CHUNKS]
            SG = [[None] * MH for _ in CHUNKS]   # (ei, ss, at, ut) per (ci, j)
            prev_h = [None] * MH
            prev_w = [TC]

            def emit_group(ci, p, j):
                w = CHUNKS[ci][1]
                pt = pspool.tile([128, TC], f32, tag="ps", name=f"ps{ci}_{j}_{p}")
                for kd in range(KD):
                    nc.tensor.matmul(
                        pt[:, 0:w],
                        w_sb[p][kd // 2][:, kd % 2:kd % 2 + 1, j * 128:(j + 1) * 128],
                        xslice(ci, kd),
                        start=(kd == 0),
                        stop=(kd == KD - 1),
                    )
                PS[ci][j][p] = pt

            # Division-free gates: Ef=e^{-zf}, Ei=e^{-zi};
            # a = (1+Ei)/(2+Ef+Ei) with 1/s2 = Exp(-Ln(ss+2)); u = 1-a.
            def stage1(ci, j):
                w = CHUNKS[ci][1]
                ps = PS[ci][j]
                ef = wk.tile([128, TC], bf16, tag="ef", name=f"ef{ci}_{j}")
                nc.scalar.activation(ef[:, 0:w], ps["f"][:, 0:w], Act.Exp, bias=b_sb["f"][:, j:j + 1], scale=-1.0)
                ei = wk.tile([128, TC], bf16, tag="ei", name=f"ei{ci}_{j}")
                nc.scalar.activation(ei[:, 0:w], ps["i"][:, 0:w], Act.Exp, bias=b_sb["i"][:, j:j + 1], scale=-1.0)
                ss = wk.tile([128, TC], bf16, tag="s2", name=f"ss{ci}_{j}")
                nc.gpsimd.tensor_tensor(ss[:, 0:w], ef[:, 0:w], ei[:, 0:w], A.add)
                SG[ci][j] = [ei, ss, None, None]

            def stage2a(ci, j):
                w = CHUNKS[ci][1]
                ei, ss, _, _ = SG[ci][j]
                ln2 = wk.tile([128, TC], f32, tag="ln2", name=f"ln{ci}_{j}")
                nc.scalar.activation(ln2[:, 0:w], ss[:, 0:w], Act.Ln, bias=two_sb[:, 0:1], scale=1.0)
                rt = wk.tile([128, TC], bf16, tag="rt", name=f"rt{ci}_{j}")
                nc.scalar.activation(rt[:, 0:w], ln2[:, 0:w], Act.Exp, bias=0.0, scale=-1.0)
                at = wk.tile([128, TC], bf16, tag="a", name=f"at{ci}_{j}", bufs=14)
                nc.vector.scalar_tensor_tensor(at[:, 0:w], ei[:, 0:w], 1.0, rt[:, 0:w], A.add, A.mult)
                ut = wk.tile([128, TC], bf16, tag="u", name=f"ut{ci}_{j}", bufs=14)
                nc.gpsimd.tensor_scalar(ut[:, 0:w], at[:, 0:w], scalar1=-1.0, scalar2=1.0, op0=A.mult, op1=A.add)
                SG[ci][j][2] = at
                SG[ci][j][3] = ut

            def stage2b(ci, j):
                tc0, w = CHUNKS[ci][0], CHUNKS[ci][1]
                ps = PS[ci][j]
                _, _, at, ut = SG[ci][j]
                bt = wk.tile([128, TC], bf16, tag="b", name=f"bt{ci}_{j}")
                nc.vector.scalar_tensor_tensor(bt[:, 0:w], ps["h"][:, 0:w], b_sb["h"][:, j:j + 1], ut[:, 0:w], A.add, A.mult)
                hh = hpool.tile([128, TC], bf16, tag=f"h{j}", name=f"hh{ci}_{j}")
                init = h0_sb[:, j:j + 1] if ci == 0 else prev_h[j][:, prev_w[0] - 1:prev_w[0]]
                nc.vector.tensor_tensor_scan(hh[:, 0:w], at[:, 0:w], bt[:, 0:w], init, op0=A.mult, op1=A.add)
                prev_h[j] = hh
                nc.gpsimd.dma_start(hT[j * 128:(j + 1) * 128, tc0:tc0 + w], hh[:, 0:w])

            def phase_a(ci):
                load_x(CHUNKS[min(ci + 1, len(CHUNKS) - 1)][2])
                if ci == 0:
                    # kd-outer so the first matmuls need only wf[kd0]+x0[kd0]
                    for p in "fi":
                        for kd in range(KD):
                            for j in range(MH):
                                w = CHUNKS[0][1]
                                if kd == 0:
                                    PS[0][j][p] = pspool.tile([128, TC], f32, tag="ps", name=f"ps0_{j}_{p}")
                                nc.tensor.matmul(
                                    PS[0][j][p][:, 0:w],
                                    w_sb[p][kd // 2][:, kd % 2:kd % 2 + 1, j * 128:(j + 1) * 128],
                                    xslice(0, kd),
                                    start=(kd == 0),
                                    stop=(kd == KD - 1),
                                )
                    for j in range(MH):
                        stage1(0, j)
                    for j in range(MH):
                        stage2a(0, j)
                else:
                    for j in range(MH):
                        emit_group(ci, "f", j)
                        emit_group(ci, "i", j)
                        stage1(ci, j)
                        if j > 0:
                            stage2a(ci, j - 1)
                    stage2a(ci, MH - 1)

            def phase_b(ci):
                for j in range(MH):
                    emit_group(ci, "h", j)
                    stage2b(ci, j)
                prev_w[0] = CHUNKS[ci][1]

            phase_a(0)
            for ci in range(1, len(CHUNKS)):
                phase_a(ci)
                phase_b(ci - 1)
            phase_b(len(CHUNKS) - 1)

    # All our ACT funcs (Exp, Ln, Identity, Copy) live in the single table
    # "natural_log_exp_and_others", but the table-load pass picks the FIRST
    # table containing each func, thrashing Exp->exp_and_others /
    # Ln->natural_log (96 swaps x 1.3us). Empty out every other table (names
    # and positions preserved, so emitted runtime table ids stay valid) so
    # first-match lands on the one shared table and a single load is emitted.
    import concourse.bacc as bacc_mod

    orig_tables = bacc_mod.get_activation_tables

    def _single_table(arch):
        tabs = orig_tables(arch)
        keep = "natural_log_exp_and_others"
        return {k: (v if k == keep else set()) for k, v in tabs.items()}

    bacc_mod.get_activation_tables = _single_table
    try:
        nc.compile()
    finally:
        bacc_mod.get_activation_tables = orig_tables
    return nc


def _get_nc():
    if "nc" not in _state:
        _state["nc"] = _build()
    return _state["nc"]


def _prep_inputs(x, h0, f_w, f_b, i_w, i_b, h_w, h_b):
    import ml_dtypes

    bf16 = ml_dtypes.bfloat16
    x = np.asarray(x, dtype=np.float32)
    h0 = np.asarray(h0, dtype=np.float32)
    # [B, D, T] -> pair-major [B, 128, KD, T] so each chunk is one DMA
    xT = x.transpose(0, 2, 1).reshape(B, KD, 128, T).transpose(0, 2, 1, 3)
    xT = np.ascontiguousarray(xT.astype(bf16))
    shared = {}
    biases = []
    for p, wgt, bias, sgn in (("f", f_w, f_b, -1.0), ("i", i_w, i_b, -1.0), ("h", h_w, h_b, 1.0)):
        wgt = np.asarray(wgt, dtype=np.float32)
        # f/i biases negated: kernel computes Exp(-pre + bias_ap), needs bias_ap = -b
        biases.append((sgn * np.asarray(bias, dtype=np.float32)).reshape(MH, 128).T)
        wP = wgt.T.reshape(KD, 128, H).transpose(1, 0, 2)   # [128, KD, H]
        shared[f"w{p}"] = np.ascontiguousarray(wP.astype(bf16))
    in_maps = []
    for b in range(B):
        m = dict(shared)
        m["xP"] = xT[b]
        h0c = h0[b, 0].reshape(MH, 128).T
        m["bcat"] = np.ascontiguousarray(np.concatenate(biases + [h0c], axis=1))
        in_maps.append(m)
    return in_maps


def kernel(x, h0, f_w, f_b, i_w, i_b, h_w, h_b, _trace=False):
    from concourse.bass_utils import run_bass_kernel_spmd

    nc = _get_nc()
    in_maps = _prep_inputs(x, h0, f_w, f_b, i_w, i_b, h_w, h_b)
    res = run_bass_kernel_spmd(nc, in_maps, core_ids=list(range(B)), trace=_trace)
    out = np.empty((B, T, H), dtype=np.float32)
    for b in range(B):
        out[b] = res.results[b]["hT"].T.astype(np.float32)
    if _trace:
        _state["last_results"] = res
    return out


# revision 12
# speedup vs baseline: 1.1419x; 1.1419x over previous
"""MinLSTM cell for Trainium2 (Bass/Tile), data-parallel over batch on 8 cores.

Per core (one batch row), software-pipelined at j-granularity, one chunk of
lag between the gate side and the h side:

  step (ci, j):  f(ci,j), i(ci,j)   - bf16 matmuls, K=768 in PSUM
                 stage1(ci,j)       - ef, ei (ACT, from PSUM), ss (Pool)
                 h(ci-1,j)          - bf16 matmuls
                 stage2b(ci-1,j)    - bt (DVE), scan (DVE), store
                 stage2a(ci,j-1)    - ln, rt (ACT), at (DVE), ut (Pool)

Only the last chunk's h/bt/scan/store trail the final gate matmul, and the
last two chunks are 256 wide to shrink that tail.

Gates are division-free via Exp/Ln from one LUT table: Ef=e^{-zf},
Ei=e^{-zi}, a = (1+Ei)/(2+Ef+Ei) with 1/s2 = Exp(-Ln(ss+2)), u = 1-a.

DMA queues (3 usable, ~88GB/s each): sync = bf16 x + even-j stores,
scalar = biases, wi, wh, odd-j stores, gpsimd = wf (loads never block the
queue; stores wait on their scan, so they're split across two queues).
Chunk 0's x and wf/wi come as per-kd tiles so the first matmul only waits
for one kd slice. Output hT [H,T] bf16; host transposes/upcasts to fp32.
"""

import sys

if "/opt/trn_rl_repo" not in sys.path:
    sys.path.insert(0, "/opt/trn_rl_repo")

import numpy as np

B, T, D, H = 8, 4096, 768, 768
TC = 512                    # steady-state time-chunk (one PSUM bank of fp32)
KD = D // 128               # 6 contraction tiles
MH = H // 128               # 6 hidden tiles
# (time offset, width, x-load index, offset within load)
CHUNKS = [(c * TC, TC, c, 0) for c in range(7)] + [(3584, 256, 7, 0), (3840, 256, 7, 256)]
XLOADS = [(c * TC, TC) for c in range(8)]

_state = {}


def _build():
    import concourse.mybir as mybir
    import concourse.tile as tile
    from concourse import bacc

    f32, bf16 = mybir.dt.float32, mybir.dt.bfloat16
    A = mybir.AluOpType
    Act = mybir.ActivationFunctionType

    nc = bacc.Bacc("TRN2", target_bir_lowering=False, debug=False, num_devices=B)

    xP = nc.dram_tensor("xP", [128, KD, T], bf16, kind="ExternalInput")
    w_d = {p: nc.dram_tensor(f"w{p}", [128, KD, H], bf16, kind="ExternalInput") for p in "fih"}
    bc_d = nc.dram_tensor("bcat", [128, 4 * MH], f32, kind="ExternalInput")
    hT = nc.dram_tensor("hT", [H, T], bf16, kind="ExternalOutput")

    with tile.TileContext(nc) as tc:
        with (
            tc.tile_pool(name="wpool", bufs=1) as wpool,
            tc.tile_pool(name="cpool", bufs=1) as cpool,
            tc.tile_pool(name="xpool", bufs=3) as xpool,
            tc.tile_pool(name="pspool", bufs=8, space="PSUM") as pspool,
            tc.tile_pool(name="wk", bufs=6) as wk,
            tc.tile_pool(name="hpool", bufs=3) as hpool,
        ):
            # Head DMAs: chunk-0 x per kd on sync; wf per kd on gpsimd;
            # biases, wi (per kd), wh (kd pairs) on scalar.
            x0k = []
            for kd in range(KD):
                t = xpool.tile([128, TC], bf16, tag=f"x0k{kd}", name=f"x0k{kd}")
                nc.sync.dma_start(t[:], xP[:, kd, 0:TC])
                x0k.append(t)
            w_sb = {p: [] for p in "fih"}
            for kd in range(KD):
                t = wpool.tile([128, 1, H], bf16, tag=f"wf{kd}", name=f"wf{kd}")
                nc.gpsimd.dma_start(t[:], w_d["f"][:, kd:kd + 1, :])
                w_sb["f"].append(t)
            bcat = cpool.tile([128, 4 * MH], f32, tag="bc")
            nc.scalar.dma_start(bcat[:], bc_d[:])
            for kd in range(KD):
                t = wpool.tile([128, 1, H], bf16, tag=f"wi{kd}", name=f"wi{kd}")
                nc.scalar.dma_start(t[:], w_d["i"][:, kd:kd + 1, :])
                w_sb["i"].append(t)
            for kp in range(KD // 2):
                t = wpool.tile([128, 2, H], bf16, tag=f"wh{kp}", name=f"wh{kp}")
                nc.scalar.dma_start(t[:], w_d["h"][:, 2 * kp:2 * kp + 2, :])
                w_sb["h"].append(t)
            b_sb = {"f": bcat[:, 0:MH], "i": bcat[:, MH:2 * MH], "h": bcat[:, 2 * MH:3 * MH]}
            h0_sb = bcat[:, 3 * MH:4 * MH]
            two_sb = cpool.tile([128, 1], f32, tag="two")
            nc.gpsimd.memset(two_sb[:], 2.0)

            def wslice(p, kd, j):
                if p == "h":
                    return w_sb["h"][kd // 2][:, kd % 2:kd % 2 + 1, j * 128:(j + 1) * 128]
                return w_sb[p][kd][:, :, j * 128:(j + 1) * 128]

            xtiles = [None] * len(XLOADS)

            def xslice(ci, kd):
                _, w, li, off = CHUNKS[ci]
                if li == 0:
                    return x0k[kd][:, 0:w]
                return xtiles[li][:, kd:kd + 1, off:off + w]

            def load_x(li):
                if li == 0 or xtiles[li] is not None:
                    return
                t0x, wx = XLOADS[li]
                xt = xpool.tile([128, KD, TC], bf16, tag="x", name=f"x{li}")
                nc.sync.dma_start(xt[:, :, 0:wx], xP[:, :, t0x:t0x + wx])
                xtiles[li] = xt

            PS = [[dict() for _ in range(MH)] for _ in CHUNKS]
            SG = [[None] * MH for _ in CHUNKS]
            prev_h = [None] * MH
            prev_w = [TC]

            def emit_group(ci, p, j):
                w = CHUNKS[ci][1]
                pt = pspool.tile([128, TC], f32, tag="ps", name=f"ps{ci}_{j}_{p}")
                for kd in range(KD):
                    nc.tensor.matmul(
                        pt[:, 0:w],
                        wslice(p, kd, j),
                        xslice(ci, kd),
                        start=(kd == 0),
                        stop=(kd == KD - 1),
                    )
                PS[ci][j][p] = pt

            def stage1(ci, j):
                w = CHUNKS[ci][1]
                ps = PS[ci][j]
                ef = wk.tile([128, TC], bf16, tag="ef", name=f"ef{ci}_{j}")
                nc.scalar.activation(ef[:, 0:w], ps["f"][:, 0:w], Act.Exp, bias=b_sb["f"][:, j:j + 1], scale=-1.0)
                ei = wk.tile([128, TC], bf16, tag="ei", name=f"ei{ci}_{j}")
                nc.scalar.activation(ei[:, 0:w], ps["i"][:, 0:w], Act.Exp, bias=b_sb["i"][:, j:j + 1], scale=-1.0)
                ss = wk.tile([128, TC], bf16, tag="s2", name=f"ss{ci}_{j}")
                nc.gpsimd.tensor_tensor(ss[:, 0:w], ef[:, 0:w], ei[:, 0:w], A.add)
                SG[ci][j] = [ei, ss, None, None]

            def stage2a(ci, j):
                w = CHUNKS[ci][1]
                ei, ss, _, _ = SG[ci][j]
                ln2 = wk.tile([128, TC], f32, tag="ln2", name=f"ln{ci}_{j}")
                nc.scalar.activation(ln2[:, 0:w], ss[:, 0:w], Act.Ln, bias=two_sb[:, 0:1], scale=1.0)
                rt = wk.tile([128, TC], bf16, tag="rt", name=f"rt{ci}_{j}")
                nc.scalar.activation(rt[:, 0:w], ln2[:, 0:w], Act.Exp, bias=0.0, scale=-1.0)
                at = wk.tile([128, TC], bf16, tag="a", name=f"at{ci}_{j}", bufs=14)
                nc.vector.scalar_tensor_tensor(at[:, 0:w], ei[:, 0:w], 1.0, rt[:, 0:w], A.add, A.mult)
                ut = wk.tile([128, TC], bf16, tag="u", name=f"ut{ci}_{j}", bufs=14)
                nc.gpsimd.tensor_scalar(ut[:, 0:w], at[:, 0:w], scalar1=-1.0, scalar2=1.0, op0=A.mult, op1=A.add)
                SG[ci][j][2] = at
                SG[ci][j][3] = ut

            def stage2b(ci, j):
                tc0, w = CHUNKS[ci][0], CHUNKS[ci][1]
                ps = PS[ci][j]
                _, _, at, ut = SG[ci][j]
                bt = wk.tile([128, TC], bf16, tag="b", name=f"bt{ci}_{j}")
                nc.vector.scalar_tensor_tensor(bt[:, 0:w], ps["h"][:, 0:w], b_sb["h"][:, j:j + 1], ut[:, 0:w], A.add, A.mult)
                hh = hpool.tile([128, TC], bf16, tag=f"h{j}", name=f"hh{ci}_{j}")
                init = h0_sb[:, j:j + 1] if ci == 0 else prev_h[j][:, prev_w[0] - 1:prev_w[0]]
                nc.vector.tensor_tensor_scan(hh[:, 0:w], at[:, 0:w], bt[:, 0:w], init, op0=A.mult, op1=A.add)
                prev_h[j] = hh
                q = nc.sync if j % 2 == 0 else nc.scalar
                q.dma_start(hT[j * 128:(j + 1) * 128, tc0:tc0 + w], hh[:, 0:w])

            # A(0): kd-outer in j-triples so the first matmuls need only
            # wf[kd0]+x0[kd0], and PSUM stays within 8 banks.
            load_x(1)
            for p in "fi":
                for jh in (range(0, 3), range(3, 6)):
                    for kd in range(KD):
                        for j in jh:
                            if kd == 0:
                                PS[0][j][p] = pspool.tile([128, TC], f32, tag="ps", name=f"ps0_{j}_{p}")
                            nc.tensor.matmul(
                                PS[0][j][p][:, 0:TC],
                                wslice(p, kd, j),
                                xslice(0, kd),
                                start=(kd == 0),
                                stop=(kd == KD - 1),
                            )
            for j in range(MH):
                stage1(0, j)
            for j in range(MH):
                stage2a(0, j)

            for ci in range(1, len(CHUNKS)):
                load_x(CHUNKS[min(ci + 1, len(CHUNKS) - 1)][2])
                for j in range(MH):
                    emit_group(ci, "f", j)
                    emit_group(ci, "i", j)
                    stage1(ci, j)
                    emit_group(ci - 1, "h", j)
                    stage2b(ci - 1, j)
                    if j > 0:
                        stage2a(ci, j - 1)
                stage2a(ci, MH - 1)
                if ci == len(CHUNKS) - 1:
                    prev_w[0] = CHUNKS[ci - 1][1]
            for j in range(MH):
                emit_group(len(CHUNKS) - 1, "h", j)
                stage2b(len(CHUNKS) - 1, j)

    # All our ACT funcs (Exp, Ln) live in the single table
    # "natural_log_exp_and_others"; empty every other table so the
    # table-load pass emits exactly one load (names/positions preserved).
    import concourse.bacc as bacc_mod

    orig_tables = bacc_mod.get_activation_tables

    def _single_table(arch):
        tabs = orig_tables(arch)
        keep = "natural_log_exp_and_others"
        return {k: (v if k == keep else set()) for k, v in tabs.items()}

    bacc_mod.get_activation_tables = _single_table
    try:
        nc.compile()
    finally:
        bacc_mod.get_activation_tables = orig_tables
    return nc


def _get_nc():
    if "nc" not in _state:
        _state["nc"] = _build()
    return _state["nc"]


def _prep_inputs(x, h0, f_w, f_b, i_w, i_b, h_w, h_b):
    import ml_dtypes

    bf16 = ml_dtypes.bfloat16
    x = np.asarray(x, dtype=np.float32)
    h0 = np.asarray(h0, dtype=np.float32)
    # [B, D, T] -> kd-major [B, 128, KD, T] so each chunk is one DMA
    xT = x.transpose(0, 2, 1).reshape(B, KD, 128, T).transpose(0, 2, 1, 3)
    xT = np.ascontiguousarray(xT.astype(bf16))
    shared = {}
    biases = []
    for p, wgt, bias, sgn in (("f", f_w, f_b, -1.0), ("i", i_w, i_b, -1.0), ("h", h_w, h_b, 1.0)):
        wgt = np.asarray(wgt, dtype=np.float32)
        # f/i biases negated: kernel computes Exp(-pre + bias_ap), needs bias_ap = -b
        biases.append((sgn * np.asarray(bias, dtype=np.float32)).reshape(MH, 128).T)
        wP = wgt.T.reshape(KD, 128, H).transpose(1, 0, 2)   # [128, KD, H]
        shared[f"w{p}"] = np.ascontiguousarray(wP.astype(bf16))
    in_maps = []
    for b in range(B):
        m = dict(shared)
        m["xP"] = xT[b]
        h0c = h0[b, 0].reshape(MH, 128).T
        m["bcat"] = np.ascontiguousarray(np.concatenate(biases + [h0c], axis=1))
        in_maps.append(m)
    return in_maps


def kernel(x, h0, f_w, f_b, i_w, i_b, h_w, h_b, _trace=False):
    from concourse.bass_utils import run_bass_kernel_spmd

    nc = _get_nc()
    in_maps = _prep_inputs(x, h0, f_w, f_b, i_w, i_b, h_w, h_b)
    res = run_bass_kernel_spmd(nc, in_maps, core_ids=list(range(B)), trace=_trace)
    out = np.empty((B, T, H), dtype=np.float32)
    for b in range(B):
        out[b] = res.results[b]["hT"].T.astype(np.float32)
    if _trace:
        _state["last_results"] = res
    return out


# revision 15
# speedup vs baseline: 1.2299x; 1.0771x over previous
"""MinLSTM cell for Trainium2 (Bass/Tile), data-parallel over batch on 8 cores.

Per core (one batch row), software-pipelined at j-granularity, one chunk of
lag between the gate side and the h side:

  step (ci, j):  f(ci,j), i(ci,j)   - bf16 matmuls, K=768 in PSUM
                 stage1(ci,j)       - ef, ei (ACT, from PSUM), ss (DVE)
                 h(ci-1,j)          - bf16 matmuls
                 stage2b(ci-1,j)    - bt (DVE), scan (DVE), store
                 stage2a(ci,j-1)    - ln, rt (ACT), at, ut (DVE)

All elementwise work sits on ACT+DVE; the Pool engine only issues the wf
load (its SBUF port is shared with DVE, so Pool compute slows DVE scans).

Only the last chunk's h/bt/scan/store trail the final gate matmul, and the
last two chunks are 256 wide to shrink that tail.

Gates are division-free via Exp/Ln from one LUT table: Ef=e^{-zf},
Ei=e^{-zi}, a = (1+Ei)/(2+Ef+Ei) with 1/s2 = Exp(-Ln(ss+2)), u = 1-a.

DMA queues (3 usable, ~88GB/s each): sync = bf16 x + even-j stores,
scalar = biases, wi, wh, odd-j stores, gpsimd = wf (loads never block the
queue; stores wait on their scan, so they're split across two queues).
Chunk 0's x and wf/wi come as per-kd tiles so the first matmul only waits
for one kd slice. Output hT [H,T] bf16; host transposes/upcasts to fp32.
"""

import sys

if "/opt/trn_rl_repo" not in sys.path:
    sys.path.insert(0, "/opt/trn_rl_repo")

import numpy as np

B, T, D, H = 8, 4096, 768, 768
TC = 512                    # steady-state time-chunk (one PSUM bank of fp32)
KD = D // 128               # 6 contraction tiles
MH = H // 128               # 6 hidden tiles
# (time offset, width, x-load index, offset within load)
CHUNKS = [(c * TC, TC, c, 0) for c in range(7)] + [(3584, 256, 7, 0), (3840, 256, 7, 256)]
XLOADS = [(c * TC, TC) for c in range(8)]

_state = {}


def _build():
    import concourse.mybir as mybir
    import concourse.tile as tile
    from concourse import bacc

    f32, bf16 = mybir.dt.float32, mybir.dt.bfloat16
    A = mybir.AluOpType
    Act = mybir.ActivationFunctionType

    nc = bacc.Bacc("TRN2", target_bir_lowering=False, debug=False, num_devices=B)

    xP = nc.dram_tensor("xP", [128, KD, T], bf16, kind="ExternalInput")
    w_d = {p: nc.dram_tensor(f"w{p}", [128, KD, H], bf16, kind="ExternalInput") for p in "fih"}
    bc_d = nc.dram_tensor("bcat", [128, 4 * MH], f32, kind="ExternalInput")
    hT = nc.dram_tensor("hT", [H, T], bf16, kind="ExternalOutput")

    with tile.TileContext(nc) as tc:
        with (
            tc.tile_pool(name="wpool", bufs=1) as wpool,
            tc.tile_pool(name="cpool", bufs=1) as cpool,
            tc.tile_pool(name="xpool", bufs=3) as xpool,
            tc.tile_pool(name="pspool", bufs=8, space="PSUM") as pspool,
            tc.tile_pool(name="wk", bufs=6) as wk,
            tc.tile_pool(name="hpool", bufs=3) as hpool,
        ):
            # Head DMAs: chunk-0 x per kd on sync; wf per kd on gpsimd;
            # biases, wi (per kd), wh (kd pairs) on scalar.
            x0k = []
            for kd in range(KD):
                t = xpool.tile([128, TC], bf16, tag=f"x0k{kd}", name=f"x0k{kd}")
                nc.sync.dma_start(t[:], xP[:, kd, 0:TC])
                x0k.append(t)
            w_sb = {p: [] for p in "fih"}
            for kd in range(KD):
                t = wpool.tile([128, 1, H], bf16, tag=f"wf{kd}", name=f"wf{kd}")
                nc.gpsimd.dma_start(t[:], w_d["f"][:, kd:kd + 1, :])
                w_sb["f"].append(t)
            bcat = cpool.tile([128, 4 * MH], f32, tag="bc")
            nc.scalar.dma_start(bcat[:], bc_d[:])
            for kd in range(KD):
                t = wpool.tile([128, 1, H], bf16, tag=f"wi{kd}", name=f"wi{kd}")
                nc.scalar.dma_start(t[:], w_d["i"][:, kd:kd + 1, :])
                w_sb["i"].append(t)
            for kp in range(KD // 2):
                t = wpool.tile([128, 2, H], bf16, tag=f"wh{kp}", name=f"wh{kp}")
                nc.scalar.dma_start(t[:], w_d["h"][:, 2 * kp:2 * kp + 2, :])
                w_sb["h"].append(t)
            b_sb = {"f": bcat[:, 0:MH], "i": bcat[:, MH:2 * MH], "h": bcat[:, 2 * MH:3 * MH]}
            h0_sb = bcat[:, 3 * MH:4 * MH]
            two_sb = cpool.tile([128, 1], f32, tag="two")
            nc.gpsimd.memset(two_sb[:], 2.0)

            def wslice(p, kd, j):
                if p == "h":
                    return w_sb["h"][kd // 2][:, kd % 2:kd % 2 + 1, j * 128:(j + 1) * 128]
                return w_sb[p][kd][:, :, j * 128:(j + 1) * 128]

            xtiles = [None] * len(XLOADS)

            def xslice(ci, kd):
                _, w, li, off = CHUNKS[ci]
                if li == 0:
                    return x0k[kd][:, 0:w]
                return xtiles[li][:, kd:kd + 1, off:off + w]

            def load_x(li):
                if li == 0 or xtiles[li] is not None:
                    return
                t0x, wx = XLOADS[li]
                xt = xpool.tile([128, KD, TC], bf16, tag="x", name=f"x{li}")
                nc.sync.dma_start(xt[:, :, 0:wx], xP[:, :, t0x:t0x + wx])
                xtiles[li] = xt

            PS = [[dict() for _ in range(MH)] for _ in CHUNKS]
            SG = [[None] * MH for _ in CHUNKS]
            prev_h = [None] * MH
            prev_w = [TC]

            def emit_group(ci, p, j):
                w = CHUNKS[ci][1]
                pt = pspool.tile([128, TC], f32, tag="ps", name=f"ps{ci}_{j}_{p}")
                for kd in range(KD):
                    nc.tensor.matmul(
                        pt[:, 0:w],
                        wslice(p, kd, j),
                        xslice(ci, kd),
                        start=(kd == 0),
                        stop=(kd == KD - 1),
                    )
                PS[ci][j][p] = pt

            def stage1(ci, j):
                w = CHUNKS[ci][1]
                ps = PS[ci][j]
                ef = wk.tile([128, TC], bf16, tag="ef", name=f"ef{ci}_{j}")
                nc.scalar.activation(ef[:, 0:w], ps["f"][:, 0:w], Act.Exp, bias=b_sb["f"][:, j:j + 1], scale=-1.0)
                ei = wk.tile([128, TC], bf16, tag="ei", name=f"ei{ci}_{j}")
                nc.scalar.activation(ei[:, 0:w], ps["i"][:, 0:w], Act.Exp, bias=b_sb["i"][:, j:j + 1], scale=-1.0)
                ss = wk.tile([128, TC], bf16, tag="s2", name=f"ss{ci}_{j}")
                nc.vector.tensor_tensor(ss[:, 0:w], ef[:, 0:w], ei[:, 0:w], A.add)
                SG[ci][j] = [ei, ss, None, None]

            def stage2a(ci, j):
                w = CHUNKS[ci][1]
                ei, ss, _, _ = SG[ci][j]
                ln2 = wk.tile([128, TC], f32, tag="ln2", name=f"ln{ci}_{j}")
                nc.scalar.activation(ln2[:, 0:w], ss[:, 0:w], Act.Ln, bias=two_sb[:, 0:1], scale=1.0)
                rt = wk.tile([128, TC], bf16, tag="rt", name=f"rt{ci}_{j}")
                nc.scalar.activation(rt[:, 0:w], ln2[:, 0:w], Act.Exp, bias=0.0, scale=-1.0)
                at = wk.tile([128, TC], bf16, tag="a", name=f"at{ci}_{j}", bufs=14)
                nc.vector.scalar_tensor_tensor(at[:, 0:w], ei[:, 0:w], 1.0, rt[:, 0:w], A.add, A.mult)
                ut = wk.tile([128, TC], bf16, tag="u", name=f"ut{ci}_{j}", bufs=14)
                nc.vector.tensor_scalar(ut[:, 0:w], at[:, 0:w], scalar1=-1.0, scalar2=1.0, op0=A.mult, op1=A.add)
                SG[ci][j][2] = at
                SG[ci][j][3] = ut

            def stage2b(ci, j):
                tc0, w = CHUNKS[ci][0], CHUNKS[ci][1]
                ps = PS[ci][j]
                _, _, at, ut = SG[ci][j]
                bt = wk.tile([128, TC], bf16, tag="b", name=f"bt{ci}_{j}")
                nc.vector.scalar_tensor_tensor(bt[:, 0:w], ps["h"][:, 0:w], b_sb["h"][:, j:j + 1], ut[:, 0:w], A.add, A.mult)
                hh = hpool.tile([128, TC], bf16, tag=f"h{j}", name=f"hh{ci}_{j}")
                init = h0_sb[:, j:j + 1] if ci == 0 else prev_h[j][:, prev_w[0] - 1:prev_w[0]]
                nc.vector.tensor_tensor_scan(hh[:, 0:w], at[:, 0:w], bt[:, 0:w], init, op0=A.mult, op1=A.add)
                prev_h[j] = hh
                q = nc.sync if j % 2 == 0 else nc.scalar
                q.dma_start(hT[j * 128:(j + 1) * 128, tc0:tc0 + w], hh[:, 0:w])

            # A(0): kd-outer in j-triples so the first matmuls need only
            # wf[kd0]+x0[kd0], and PSUM stays within 8 banks.
            load_x(1)
            for p in "fi":
                for jh in (range(0, 3), range(3, 6)):
                    for kd in range(KD):
                        for j in jh:
                            if kd == 0:
                                PS[0][j][p] = pspool.tile([128, TC], f32, tag="ps", name=f"ps0_{j}_{p}")
                            nc.tensor.matmul(
                                PS[0][j][p][:, 0:TC],
                                wslice(p, kd, j),
                                xslice(0, kd),
                                start=(kd == 0),
                                stop=(kd == KD - 1),
                            )
            for j in range(MH):
                stage1(0, j)
            for j in range(MH):
                stage2a(0, j)

            for ci in range(1, len(CHUNKS)):
                load_x(CHUNKS[min(ci + 1, len(CHUNKS) - 1)][2])
                for j in range(MH):
                    emit_group(ci, "f", j)
                    emit_group(ci, "i", j)
                    stage1(ci, j)
                    emit_group(ci - 1, "h", j)
                    stage2b(ci - 1, j)
                    if j > 0:
                        stage2a(ci, j - 1)
                stage2a(ci, MH - 1)
                if ci == len(CHUNKS) - 1:
                    prev_w[0] = CHUNKS[ci - 1][1]
            for j in range(MH):
                emit_group(len(CHUNKS) - 1, "h", j)
                stage2b(len(CHUNKS) - 1, j)

    # All our ACT funcs (Exp, Ln) live in the single table
    # "natural_log_exp_and_others"; empty every other table so the
    # table-load pass emits exactly one load (names/positions preserved).
    import concourse.bacc as bacc_mod

    orig_tables = bacc_mod.get_activation_tables

    def _single_table(arch):
        tabs = orig_tables(arch)
        keep = "natural_log_exp_and_others"
        return {k: (v if k == keep else set()) for k, v in tabs.items()}

    bacc_mod.get_activation_tables = _single_table
    try:
        nc.compile()
    finally:
        bacc_mod.get_activation_tables = orig_tables
    return nc


def _get_nc():
    if "nc" not in _state:
        _state["nc"] = _build()
    return _state["nc"]


def _prep_inputs(x, h0, f_w, f_b, i_w, i_b, h_w, h_b):
    import ml_dtypes

    bf16 = ml_dtypes.bfloat16
    x = np.asarray(x, dtype=np.float32)
    h0 = np.asarray(h0, dtype=np.float32)
    # [B, D, T] -> kd-major [B, 128, KD, T] so each chunk is one DMA
    xT = x.transpose(0, 2, 1).reshape(B, KD, 128, T).transpose(0, 2, 1, 3)
    xT = np.ascontiguousarray(xT.astype(bf16))
    shared = {}
    biases = []
    for p, wgt, bias, sgn in (("f", f_w, f_b, -1.0), ("i", i_w, i_b, -1.0), ("h", h_w, h_b, 1.0)):
        wgt = np.asarray(wgt, dtype=np.float32)
        # f/i biases negated: kernel computes Exp(-pre + bias_ap), needs bias_ap = -b
        biases.append((sgn * np.asarray(bias, dtype=np.float32)).reshape(MH, 128).T)
        wP = wgt.T.reshape(KD, 128, H).transpose(1, 0, 2)   # [128, KD, H]
        shared[f"w{p}"] = np.ascontiguousarray(wP.astype(bf16))
    in_maps = []
    for b in range(B):
        m = dict(shared)
        m["xP"] = xT[b]
        h0c = h0[b, 0].reshape(MH, 128).T
        m["bcat"] = np.ascontiguousarray(np.concatenate(biases + [h0c], axis=1))
        in_maps.append(m)
    return in_maps


def kernel(x, h0, f_w, f_b, i_w, i_b, h_w, h_b, _trace=False):
    from concourse.bass_utils import run_bass_kernel_spmd

    nc = _get_nc()
    in_maps = _prep_inputs(x, h0, f_w, f_b, i_w, i_b, h_w, h_b)
    res = run_bass_kernel_spmd(nc, in_maps, core_ids=list(range(B)), trace=_trace)
    out = np.empty((B, T, H), dtype=np.float32)
    for b in range(B):
        out[b] = res.results[b]["hT"].T.astype(np.float32)
    if _trace:
        _state["last_results"] = res
    return out


# revision 17
# speedup vs baseline: 1.2311x; 1.0010x over previous
"""MinLSTM cell for Trainium2 (Bass/Tile), data-parallel over batch on 8 cores.

Per core (one batch row), software-pipelined at j-granularity, one chunk of
lag between the gate side and the h side:

  step (ci, j):  f(ci,j), i(ci,j)   - bf16 matmuls, K=768 in PSUM
                 stage1(ci,j)       - ef, ei (ACT, from PSUM), ss (DVE)
                 h(ci-1,j)          - bf16 matmuls
                 stage2b(ci-1,j)    - bt (DVE), scan (DVE), store
                 stage2a(ci,j-1)    - ln, rt (ACT), at, ut (DVE)

All elementwise work sits on ACT+DVE; the Pool engine only issues the wf
load (its SBUF port is shared with DVE, so Pool compute slows DVE scans).

Only the last chunk's h/bt/scan/store trail the final gate matmul, and the
last two chunks are 256 wide to shrink that tail.

Gates are division-free via Exp/Ln from one LUT table: Ef=e^{-zf},
Ei=e^{-zi}, a = (1+Ei)/(2+Ef+Ei) with 1/s2 = Exp(-Ln(ss+2)), u = 1-a.

DMA queues (3 usable, ~88GB/s each): sync = bf16 x + even-j stores,
scalar = biases, wi, wh, odd-j stores, gpsimd = wf (loads never block the
queue; stores wait on their scan, so they're split across two queues).
Chunk 0's x and wf/wi come as per-kd tiles so the first matmul only waits
for one kd slice. Output hT [H,T] bf16; host transposes/upcasts to fp32.
"""

import sys

if "/opt/trn_rl_repo" not in sys.path:
    sys.path.insert(0, "/opt/trn_rl_repo")

import numpy as np

B, T, D, H = 8, 4096, 768, 768
TC = 512                    # steady-state time-chunk (one PSUM bank of fp32)
KD = D // 128               # 6 contraction tiles
MH = H // 128               # 6 hidden tiles
# (time offset, width, x-load index, offset within load)
CHUNKS = [(c * TC, TC, c, 0) for c in range(7)] + [(3584, 256, 7, 0), (3840, 256, 7, 256)]
XLOADS = [(c * TC, TC) for c in range(8)]

_state = {}


def _build():
    import concourse.mybir as mybir
    import concourse.tile as tile
    from concourse import bacc

    f32, bf16 = mybir.dt.float32, mybir.dt.bfloat16
    A = mybir.AluOpType
    Act = mybir.ActivationFunctionType

    nc = bacc.Bacc("TRN2", target_bir_lowering=False, debug=False, num_devices=B)

    xP = nc.dram_tensor("xP", [128, KD, T], bf16, kind="ExternalInput")
    w_d = {p: nc.dram_tensor(f"w{p}", [128, KD, H], bf16, kind="ExternalInput") for p in "fih"}
    bc_d = nc.dram_tensor("bcat", [128, 4 * MH], f32, kind="ExternalInput")
    hT = nc.dram_tensor("hT", [H, T], bf16, kind="ExternalOutput")

    with tile.TileContext(nc) as tc:
        with (
            tc.tile_pool(name="wpool", bufs=1) as wpool,
            tc.tile_pool(name="cpool", bufs=1) as cpool,
            tc.tile_pool(name="xpool", bufs=3) as xpool,
            tc.tile_pool(name="pspool", bufs=8, space="PSUM") as pspool,
            tc.tile_pool(name="wk", bufs=6) as wk,
            tc.tile_pool(name="hpool", bufs=3) as hpool,
        ):
            # Head DMAs: chunk-0 x per kd on sync; wf per kd on gpsimd;
            # biases, wi (per kd), wh (kd pairs) on scalar.
            x0k = []
            for kd in range(KD):
                t = xpool.tile([128, TC], bf16, tag=f"x0k{kd}", name=f"x0k{kd}")
                nc.sync.dma_start(t[:], xP[:, kd, 0:TC])
                x0k.append(t)
            w_sb = {p: [] for p in "fih"}
            for kd in range(KD):
                t = wpool.tile([128, 1, H], bf16, tag=f"wf{kd}", name=f"wf{kd}")
                nc.gpsimd.dma_start(t[:], w_d["f"][:, kd:kd + 1, :])
                w_sb["f"].append(t)
            bcat = cpool.tile([128, 4 * MH], f32, tag="bc")
            nc.scalar.dma_start(bcat[:], bc_d[:])
            for kd in range(KD):
                t = wpool.tile([128, 1, H], bf16, tag=f"wi{kd}", name=f"wi{kd}")
                nc.scalar.dma_start(t[:], w_d["i"][:, kd:kd + 1, :])
                w_sb["i"].append(t)
            for kp in range(KD // 2):
                t = wpool.tile([128, 2, H], bf16, tag=f"wh{kp}", name=f"wh{kp}")
                nc.scalar.dma_start(t[:], w_d["h"][:, 2 * kp:2 * kp + 2, :])
                w_sb["h"].append(t)
            b_sb = {"f": bcat[:, 0:MH], "i": bcat[:, MH:2 * MH], "h": bcat[:, 2 * MH:3 * MH]}
            h0_sb = bcat[:, 3 * MH:4 * MH]
            two_sb = cpool.tile([128, 1], f32, tag="two")
            nc.gpsimd.memset(two_sb[:], 2.0)

            def wslice(p, kd, j):
                if p == "h":
                    return w_sb["h"][kd // 2][:, kd % 2:kd % 2 + 1, j * 128:(j + 1) * 128]
                return w_sb[p][kd][:, :, j * 128:(j + 1) * 128]

            xtiles = [None] * len(XLOADS)

            def xslice(ci, kd):
                _, w, li, off = CHUNKS[ci]
                if li == 0:
                    return x0k[kd][:, 0:w]
                return xtiles[li][:, kd:kd + 1, off:off + w]

            def load_x(li):
                if li == 0 or xtiles[li] is not None:
                    return
                t0x, wx = XLOADS[li]
                xt = xpool.tile([128, KD, TC], bf16, tag="x", name=f"x{li}")
                nc.sync.dma_start(xt[:, :, 0:wx], xP[:, :, t0x:t0x + wx])
                xtiles[li] = xt

            PS = [[dict() for _ in range(MH)] for _ in CHUNKS]
            SG = [[None] * MH for _ in CHUNKS]
            prev_h = [None] * MH
            prev_w = [TC]

            def emit_group(ci, p, j):
                w = CHUNKS[ci][1]
                pt = pspool.tile([128, TC], f32, tag="ps", name=f"ps{ci}_{j}_{p}")
                for kd in range(KD):
                    nc.tensor.matmul(
                        pt[:, 0:w],
                        wslice(p, kd, j),
                        xslice(ci, kd),
                        start=(kd == 0),
                        stop=(kd == KD - 1),
                    )
                PS[ci][j][p] = pt

            def stage1(ci, j):
                w = CHUNKS[ci][1]
                ps = PS[ci][j]
                ef = wk.tile([128, TC], bf16, tag="ef", name=f"ef{ci}_{j}")
                nc.scalar.activation(ef[:, 0:w], ps["f"][:, 0:w], Act.Exp, bias=b_sb["f"][:, j:j + 1], scale=-1.0)
                ei = wk.tile([128, TC], bf16, tag="ei", name=f"ei{ci}_{j}")
                nc.scalar.activation(ei[:, 0:w], ps["i"][:, 0:w], Act.Exp, bias=b_sb["i"][:, j:j + 1], scale=-1.0)
                ss = wk.tile([128, TC], bf16, tag="s2", name=f"ss{ci}_{j}")
                nc.vector.tensor_tensor(ss[:, 0:w], ef[:, 0:w], ei[:, 0:w], A.add)
                SG[ci][j] = [ei, ss, None, None]

            def stage2a(ci, j):
                w = CHUNKS[ci][1]
                ei, ss, _, _ = SG[ci][j]
                ln2 = wk.tile([128, TC], f32, tag="ln2", name=f"ln{ci}_{j}")
                nc.scalar.activation(ln2[:, 0:w], ss[:, 0:w], Act.Ln, bias=two_sb[:, 0:1], scale=1.0)
                rt = wk.tile([128, TC], bf16, tag="rt", name=f"rt{ci}_{j}")
                nc.scalar.activation(rt[:, 0:w], ln2[:, 0:w], Act.Exp, bias=0.0, scale=-1.0)
                at = wk.tile([128, TC], bf16, tag="a", name=f"at{ci}_{j}", bufs=14)
                nc.vector.scalar_tensor_tensor(at[:, 0:w], ei[:, 0:w], 1.0, rt[:, 0:w], A.add, A.mult)
                ut = wk.tile([128, TC], bf16, tag="u", name=f"ut{ci}_{j}", bufs=14)
                nc.vector.tensor_scalar(ut[:, 0:w], at[:, 0:w], scalar1=-1.0, scalar2=1.0, op0=A.mult, op1=A.add)
                SG[ci][j][2] = at
                SG[ci][j][3] = ut

            def stage2b(ci, j):
                tc0, w = CHUNKS[ci][0], CHUNKS[ci][1]
                ps = PS[ci][j]
                _, _, at, ut = SG[ci][j]
                last = ci == len(CHUNKS) - 1
                bt = wk.tile([128, TC], bf16, tag="b", name=f"bt{ci}_{j}")
                nc.vector.scalar_tensor_tensor(bt[:, 0:w], ps["h"][:, 0:w], b_sb["h"][:, j:j + 1], ut[:, 0:w], A.add, A.mult)
                q = nc.sync if j % 2 == 0 else nc.scalar
                if last:
                    # Scan into the second-to-last chunk's tile at its tail so
                    # the final 512 steps store as ONE 1KB-line DMA per j
                    # (256-wide stores have 512B lines and drain ~11us).
                    hh = prev_h[j]
                    off = prev_w[0]
                    nc.vector.tensor_tensor_scan(
                        hh[:, off:off + w], at[:, 0:w], bt[:, 0:w],
                        hh[:, off - 1:off], op0=A.mult, op1=A.add)
                    q.dma_start(hT[j * 128:(j + 1) * 128, tc0 - off:tc0 + w], hh[:, 0:off + w])
                    return
                hh = hpool.tile([128, TC], bf16, tag=f"h{j}", name=f"hh{ci}_{j}")
                init = h0_sb[:, j:j + 1] if ci == 0 else prev_h[j][:, prev_w[0] - 1:prev_w[0]]
                nc.vector.tensor_tensor_scan(hh[:, 0:w], at[:, 0:w], bt[:, 0:w], init, op0=A.mult, op1=A.add)
                prev_h[j] = hh
                if ci != len(CHUNKS) - 2:
                    q.dma_start(hT[j * 128:(j + 1) * 128, tc0:tc0 + w], hh[:, 0:w])

            # A(0): kd-outer in j-triples so the first matmuls need only
            # wf[kd0]+x0[kd0], and PSUM stays within 8 banks.
            load_x(1)
            for p in "fi":
                for jh in (range(0, 3), range(3, 6)):
                    for kd in range(KD):
                        for j in jh:
                            if kd == 0:
                                PS[0][j][p] = pspool.tile([128, TC], f32, tag="ps", name=f"ps0_{j}_{p}")
                            nc.tensor.matmul(
                                PS[0][j][p][:, 0:TC],
                                wslice(p, kd, j),
                                xslice(0, kd),
                                start=(kd == 0),
                                stop=(kd == KD - 1),
                            )
            for j in range(MH):
                stage1(0, j)
            for j in range(MH):
                stage2a(0, j)

            for ci in range(1, len(CHUNKS)):
                load_x(CHUNKS[min(ci + 1, len(CHUNKS) - 1)][2])
                zero_lag = ci == len(CHUNKS) - 1
                for j in range(MH):
                    emit_group(ci, "f", j)
                    emit_group(ci, "i", j)
                    stage1(ci, j)
                    if zero_lag:
                        # final chunk: no pipeline lag so its gate chains
                        # clear the engines before the closing h-phase
                        stage2a(ci, j)
                    emit_group(ci - 1, "h", j)
                    stage2b(ci - 1, j)
                    if not zero_lag and j > 0:
                        stage2a(ci, j - 1)
                if not zero_lag:
                    stage2a(ci, MH - 1)
                if ci == len(CHUNKS) - 1:
                    prev_w[0] = CHUNKS[ci - 1][1]
            for j in range(MH):
                emit_group(len(CHUNKS) - 1, "h", j)
                stage2b(len(CHUNKS) - 1, j)

    # All our ACT funcs (Exp, Ln) live in the single table
    # "natural_log_exp_and_others"; empty every other table so the
    # table-load pass emits exactly one load (names/positions preserved).
    import concourse.bacc as bacc_mod

    orig_tables = bacc_mod.get_activation_tables

    def _single_table(arch):
        tabs = orig_tables(arch)
        keep = "natural_log_exp_and_others"
        return {k: (v if k == keep else set()) for k, v in tabs.items()}

    bacc_mod.get_activation_tables = _single_table
    try:
        nc.compile()
    finally:
        bacc_mod.get_activation_tables = orig_tables
    return nc


def _get_nc():
    if "nc" not in _state:
        _state["nc"] = _build()
    return _state["nc"]


def _prep_inputs(x, h0, f_w, f_b, i_w, i_b, h_w, h_b):
    import ml_dtypes

    bf16 = ml_dtypes.bfloat16
    x = np.asarray(x, dtype=np.float32)
    h0 = np.asarray(h0, dtype=np.float32)
    # [B, D, T] -> kd-major [B, 128, KD, T] so each chunk is one DMA
    xT = x.transpose(0, 2, 1).reshape(B, KD, 128, T).transpose(0, 2, 1, 3)
    xT = np.ascontiguousarray(xT.astype(bf16))
    shared = {}
    biases = []
    for p, wgt, bias, sgn in (("f", f_w, f_b, -1.0), ("i", i_w, i_b, -1.0), ("h", h_w, h_b, 1.0)):
        wgt = np.asarray(wgt, dtype=np.float32)
        # f/i biases negated: kernel computes Exp(-pre + bias_ap), needs bias_ap = -b
        biases.append((sgn * np.asarray(bias, dtype=np.float32)).reshape(MH, 128).T)
        wP = wgt.T.reshape(KD, 128, H).transpose(1, 0, 2)   # [128, KD, H]
        shared[f"w{p}"] = np.ascontiguousarray(wP.astype(bf16))
    in_maps = []
    for b in range(B):
        m = dict(shared)
        m["xP"] = xT[b]
        h0c = h0[b, 0].reshape(MH, 128).T
        m["bcat"] = np.ascontiguousarray(np.concatenate(biases + [h0c], axis=1))
        in_maps.append(m)
    return in_maps


def kernel(x, h0, f_w, f_b, i_w, i_b, h_w, h_b, _trace=False):
    from concourse.bass_utils import run_bass_kernel_spmd

    nc = _get_nc()
    in_maps = _prep_inputs(x, h0, f_w, f_b, i_w, i_b, h_w, h_b)
    res = run_bass_kernel_spmd(nc, in_maps, core_ids=list(range(B)), trace=_trace)
    out = np.empty((B, T, H), dtype=np.float32)
    for b in range(B):
        out[b] = res.results[b]["hT"].T.astype(np.float32)
    if _trace:
        _state["last_results"] = res
    return out


# revision 22
# speedup vs baseline: 1.2315x; 1.0003x over previous
"""MinLSTM cell for Trainium2 (Bass/Tile), data-parallel over batch on 8 cores.

Per core (one batch row), software-pipelined at j-granularity, one chunk of
lag between the gate side and the h side:

  step (ci, j):  f(ci,j), i(ci,j)   - bf16 matmuls, K=768 in PSUM
                 stage1(ci,j)       - ef, ei (ACT, from PSUM), ss (DVE)
                 h(ci-1,j)          - bf16 matmuls
                 stage2b(ci-1,j)    - bt (DVE), scan (DVE), store
                 stage2a(ci,j-1)    - ln, rt (ACT), at, ut (DVE)

All elementwise work sits on ACT+DVE; the Pool engine only issues the wf
load (its SBUF port is shared with DVE, so Pool compute slows DVE scans).

The last chunk runs its gate chain with zero pipeline lag and folds half of
its h-groups into the final gate iteration, so only EPILOG h/bt/scan/store
chains trail the final gate matmul.

Gates are division-free via Exp/Ln from one LUT table: Ef=e^{-zf},
Ei=e^{-zi}, a = (1+Ei)/(2+Ef+Ei) with 1/s2 = Exp(-Ln(ss+2)), u = 1-a.

DMA queues (3 usable, ~88GB/s each): sync = bf16 x + even-j stores,
scalar = biases, wi, wh, odd-j stores, gpsimd = wf (loads never block the
queue; stores wait on their scan, so they're split across two queues).
Chunk 0's x and wf/wi come as per-kd tiles so the first matmul only waits
for one kd slice. Output hT [H,T] bf16; host transposes/upcasts to fp32.
"""

import sys

if "/opt/trn_rl_repo" not in sys.path:
    sys.path.insert(0, "/opt/trn_rl_repo")

import numpy as np

B, T, D, H = 8, 4096, 768, 768
TC = 512                    # steady-state time-chunk (one PSUM bank of fp32)
KD = D // 128               # 6 contraction tiles
MH = H // 128               # 6 hidden tiles
# (time offset, width, x-load index, offset within load)
CHUNKS = [(c * TC, TC, c, 0) for c in range(8)]
XLOADS = [(c * TC, TC) for c in range(8)]
EPILOG = 3   # h-groups of the last chunk folded into the last gate iteration

_state = {}


def _build():
    import concourse.mybir as mybir
    import concourse.tile as tile
    from concourse import bacc

    f32, bf16 = mybir.dt.float32, mybir.dt.bfloat16
    A = mybir.AluOpType
    Act = mybir.ActivationFunctionType

    nc = bacc.Bacc("TRN2", target_bir_lowering=False, debug=False, num_devices=B)

    xP = nc.dram_tensor("xP", [128, KD, T], bf16, kind="ExternalInput")
    w_d = {p: nc.dram_tensor(f"w{p}", [128, KD, H], bf16, kind="ExternalInput") for p in "fih"}
    bc_d = nc.dram_tensor("bcat", [128, 4 * MH], f32, kind="ExternalInput")
    hT = nc.dram_tensor("hT", [H, T], bf16, kind="ExternalOutput")

    with tile.TileContext(nc) as tc:
        with (
            tc.tile_pool(name="wpool", bufs=1) as wpool,
            tc.tile_pool(name="cpool", bufs=1) as cpool,
            tc.tile_pool(name="xpool", bufs=3) as xpool,
            tc.tile_pool(name="pspool", bufs=8, space="PSUM") as pspool,
            tc.tile_pool(name="wk", bufs=6) as wk,
            tc.tile_pool(name="hpool", bufs=3) as hpool,
        ):
            # Head DMAs: chunk-0 x per kd on sync; wf per kd on gpsimd;
            # biases, wi (per kd), wh (kd pairs) on scalar.
            x0k = []
            for kd in range(KD):
                t = xpool.tile([128, TC], bf16, tag=f"x0k{kd}", name=f"x0k{kd}")
                nc.sync.dma_start(t[:], xP[:, kd, 0:TC])
                x0k.append(t)
            w_sb = {p: [] for p in "fih"}
            for kd in range(KD):
                t = wpool.tile([128, 1, H], bf16, tag=f"wf{kd}", name=f"wf{kd}")
                nc.gpsimd.dma_start(t[:], w_d["f"][:, kd:kd + 1, :])
                w_sb["f"].append(t)
            bcat = cpool.tile([128, 4 * MH], f32, tag="bc")
            nc.scalar.dma_start(bcat[:], bc_d[:])
            for kd in range(KD):
                t = wpool.tile([128, 1, H], bf16, tag=f"wi{kd}", name=f"wi{kd}")
                nc.scalar.dma_start(t[:], w_d["i"][:, kd:kd + 1, :])
                w_sb["i"].append(t)
            for kp in range(KD // 2):
                t = wpool.tile([128, 2, H], bf16, tag=f"wh{kp}", name=f"wh{kp}")
                nc.scalar.dma_start(t[:], w_d["h"][:, 2 * kp:2 * kp + 2, :])
                w_sb["h"].append(t)
            b_sb = {"f": bcat[:, 0:MH], "i": bcat[:, MH:2 * MH], "h": bcat[:, 2 * MH:3 * MH]}
            h0_sb = bcat[:, 3 * MH:4 * MH]
            two_sb = cpool.tile([128, 1], f32, tag="two")
            nc.gpsimd.memset(two_sb[:], 2.0)

            def wslice(p, kd, j):
                if p == "h":
                    return w_sb["h"][kd // 2][:, kd % 2:kd % 2 + 1, j * 128:(j + 1) * 128]
                return w_sb[p][kd][:, :, j * 128:(j + 1) * 128]

            xtiles = [None] * len(XLOADS)

            def xslice(ci, kd):
                _, w, li, off = CHUNKS[ci]
                if li == 0:
                    return x0k[kd][:, 0:w]
                return xtiles[li][:, kd:kd + 1, off:off + w]

            def load_x(li):
                if li == 0 or xtiles[li] is not None:
                    return
                t0x, wx = XLOADS[li]
                xt = xpool.tile([128, KD, TC], bf16, tag="x", name=f"x{li}")
                nc.sync.dma_start(xt[:, :, 0:wx], xP[:, :, t0x:t0x + wx])
                xtiles[li] = xt

            PS = [[dict() for _ in range(MH)] for _ in CHUNKS]
            SG = [[None] * MH for _ in CHUNKS]
            prev_h = [None] * MH


            def emit_group(ci, p, j):
                w = CHUNKS[ci][1]
                pt = pspool.tile([128, TC], f32, tag="ps", name=f"ps{ci}_{j}_{p}")
                for kd in range(KD):
                    nc.tensor.matmul(
                        pt[:, 0:w],
                        wslice(p, kd, j),
                        xslice(ci, kd),
                        start=(kd == 0),
                        stop=(kd == KD - 1),
                    )
                PS[ci][j][p] = pt

            def stage1(ci, j):
                w = CHUNKS[ci][1]
                ps = PS[ci][j]
                ef = wk.tile([128, TC], bf16, tag="ef", name=f"ef{ci}_{j}")
                nc.scalar.activation(ef[:, 0:w], ps["f"][:, 0:w], Act.Exp, bias=b_sb["f"][:, j:j + 1], scale=-1.0)
                ei = wk.tile([128, TC], bf16, tag="ei", name=f"ei{ci}_{j}")
                nc.scalar.activation(ei[:, 0:w], ps["i"][:, 0:w], Act.Exp, bias=b_sb["i"][:, j:j + 1], scale=-1.0)
                ss = wk.tile([128, TC], bf16, tag="s2", name=f"ss{ci}_{j}")
                nc.vector.tensor_tensor(ss[:, 0:w], ef[:, 0:w], ei[:, 0:w], A.add)
                SG[ci][j] = [ei, ss, None, None]

            def stage2a(ci, j):
                w = CHUNKS[ci][1]
                ei, ss, _, _ = SG[ci][j]
                ln2 = wk.tile([128, TC], f32, tag="ln2", name=f"ln{ci}_{j}")
                nc.scalar.activation(ln2[:, 0:w], ss[:, 0:w], Act.Ln, bias=two_sb[:, 0:1], scale=1.0)
                rt = wk.tile([128, TC], bf16, tag="rt", name=f"rt{ci}_{j}")
                nc.scalar.activation(rt[:, 0:w], ln2[:, 0:w], Act.Exp, bias=0.0, scale=-1.0)
                at = wk.tile([128, TC], bf16, tag="a", name=f"at{ci}_{j}", bufs=14)
                nc.vector.scalar_tensor_tensor(at[:, 0:w], ei[:, 0:w], 1.0, rt[:, 0:w], A.add, A.mult)
                ut = wk.tile([128, TC], bf16, tag="u", name=f"ut{ci}_{j}", bufs=14)
                nc.vector.tensor_scalar(ut[:, 0:w], at[:, 0:w], scalar1=-1.0, scalar2=1.0, op0=A.mult, op1=A.add)
                SG[ci][j][2] = at
                SG[ci][j][3] = ut

            def stage2b(ci, j):
                tc0, w = CHUNKS[ci][0], CHUNKS[ci][1]
                ps = PS[ci][j]
                _, _, at, ut = SG[ci][j]
                bt = wk.tile([128, TC], bf16, tag="b", name=f"bt{ci}_{j}")
                nc.vector.scalar_tensor_tensor(bt[:, 0:w], ps["h"][:, 0:w], b_sb["h"][:, j:j + 1], ut[:, 0:w], A.add, A.mult)
                hh = hpool.tile([128, TC], bf16, tag=f"h{j}", name=f"hh{ci}_{j}")
                init = h0_sb[:, j:j + 1] if ci == 0 else prev_h[j][:, TC - 1:TC]
                nc.vector.tensor_tensor_scan(hh[:, 0:w], at[:, 0:w], bt[:, 0:w], init, op0=A.mult, op1=A.add)
                prev_h[j] = hh
                q = (nc.sync, nc.scalar, nc.gpsimd)[j % 3]
                q.dma_start(hT[j * 128:(j + 1) * 128, tc0:tc0 + w], hh[:, 0:w])

            # A(0): kd-outer in j-triples so the first matmuls need only
            # wf[kd0]+x0[kd0], and PSUM stays within 8 banks.
            load_x(1)
            for p in "fi":
                for jh in (range(0, 3), range(3, 6)):
                    for kd in range(KD):
                        for j in jh:
                            if kd == 0:
                                PS[0][j][p] = pspool.tile([128, TC], f32, tag="ps", name=f"ps0_{j}_{p}")
                            nc.tensor.matmul(
                                PS[0][j][p][:, 0:TC],
                                wslice(p, kd, j),
                                xslice(0, kd),
                                start=(kd == 0),
                                stop=(kd == KD - 1),
                            )
            for j in range(MH):
                stage1(0, j)
            for j in range(MH):
                stage2a(0, j)

            last = len(CHUNKS) - 1
            for ci in range(1, len(CHUNKS)):
                load_x(CHUNKS[min(ci + 1, last)][2])
                zero_lag = ci == last
                for j in range(MH):
                    emit_group(ci, "f", j)
                    emit_group(ci, "i", j)
                    stage1(ci, j)
                    if zero_lag:
                        # final chunk: no pipeline lag so its gate chains
                        # clear the engines before the closing h-phase
                        stage2a(ci, j)
                    emit_group(ci - 1, "h", j)
                    stage2b(ci - 1, j)
                    if zero_lag and j >= MH - EPILOG:
                        # fold the last chunk's first h-groups in here so
                        # only EPILOG scans trail the final matmul
                        jj = j - (MH - EPILOG)
                        emit_group(last, "h", jj)
                        stage2b(last, jj)
                    if not zero_lag and j > 0:
                        stage2a(ci, j - 1)
                if not zero_lag:
                    stage2a(ci, MH - 1)
            for j in range(MH - EPILOG, MH):
                emit_group(last, "h", j)
                stage2b(last, j)

    # All our ACT funcs (Exp, Ln) live in the single table
    # "natural_log_exp_and_others"; empty every other table so the
    # table-load pass emits exactly one load (names/positions preserved).
    import concourse.bacc as bacc_mod

    orig_tables = bacc_mod.get_activation_tables

    def _single_table(arch):
        tabs = orig_tables(arch)
        keep = "natural_log_exp_and_others"
        return {k: (v if k == keep else set()) for k, v in tabs.items()}

    bacc_mod.get_activation_tables = _single_table
    try:
        nc.compile()
    finally:
        bacc_mod.get_activation_tables = orig_tables
    return nc


def _get_nc():
    if "nc" not in _state:
        _state["nc"] = _build()
    return _state["nc"]


def _prep_inputs(x, h0, f_w, f_b, i_w, i_b, h_w, h_b):
    import ml_dtypes

    bf16 = ml_dtypes.bfloat16
    x = np.asarray(x, dtype=np.float32)
    h0 = np.asarray(h0, dtype=np.float32)
    # [B, D, T] -> kd-major [B, 128, KD, T] so each chunk is one DMA
    xT = x.transpose(0, 2, 1).reshape(B, KD, 128, T).transpose(0, 2, 1, 3)
    xT = np.ascontiguousarray(xT.astype(bf16))
    shared = {}
    biases = []
    for p, wgt, bias, sgn in (("f", f_w, f_b, -1.0), ("i", i_w, i_b, -1.0), ("h", h_w, h_b, 1.0)):
        wgt = np.asarray(wgt, dtype=np.float32)
        # f/i biases negated: kernel computes Exp(-pre + bias_ap), needs bias_ap = -b
        biases.append((sgn * np.asarray(bias, dtype=np.float32)).reshape(MH, 128).T)
        wP = wgt.T.reshape(KD, 128, H).transpose(1, 0, 2)   # [128, KD, H]
        shared[f"w{p}"] = np.ascontiguousarray(wP.astype(bf16))
    in_maps = []
    for b in range(B):
        m = dict(shared)
        m["xP"] = xT[b]
        h0c = h0[b, 0].reshape(MH, 128).T
        m["bcat"] = np.ascontiguousarray(np.concatenate(biases + [h0c], axis=1))
        in_maps.append(m)
    return in_maps


def kernel(x, h0, f_w, f_b, i_w, i_b, h_w, h_b, _trace=False):
    from concourse.bass_utils import run_bass_kernel_spmd

    nc = _get_nc()
    in_maps = _prep_inputs(x, h0, f_w, f_b, i_w, i_b, h_w, h_b)
    res = run_bass_kernel_spmd(nc, in_maps, core_ids=list(range(B)), trace=_trace)
    out = np.empty((B, T, H), dtype=np.float32)
    for b in range(B):
        out[b] = res.results[b]["hT"].T.astype(np.float32)
    if _trace:
        _state["last_results"] = res
    return out


# revision 24
# speedup vs baseline: 1.2320x; 1.0004x over previous
"""MinLSTM cell for Trainium2 (Bass/Tile), data-parallel over batch on 8 cores.

Per core (one batch row), software-pipelined at j-granularity, one chunk of
lag between the gate side and the h side:

  step (ci, j):  f(ci,j), i(ci,j)   - bf16 matmuls, K=768 in PSUM
                 stage1(ci,j)       - ef, ei (ACT, from PSUM), ss (DVE)
                 h(ci-1,j)          - bf16 matmuls
                 stage2b(ci-1,j)    - bt (DVE), scan (DVE), store
                 stage2a(ci,j-1)    - ln, rt (ACT), at, ut (DVE)

All elementwise work sits on ACT+DVE; the Pool engine only issues the wf
load (its SBUF port is shared with DVE, so Pool compute slows DVE scans).

The last chunk runs its gate chain with zero pipeline lag and folds half of
its h-groups into the final gate iteration, so only EPILOG h/bt/scan/store
chains trail the final gate matmul.

Gates are division-free via Exp/Ln from one LUT table: Ef=e^{-zf},
Ei=e^{-zi}, a = (1+Ei)/(2+Ef+Ei) with 1/s2 = Exp(-Ln(ss+2)), u = 1-a.

DMA queues (3 usable, ~88GB/s each): sync = bf16 x + even-j stores,
scalar = biases, wi, wh, odd-j stores, gpsimd = wf (loads never block the
queue; stores wait on their scan, so they're split across two queues).
Chunk 0's x and wf/wi come as per-kd tiles so the first matmul only waits
for one kd slice. Output hT [H,T] bf16; host transposes/upcasts to fp32.
"""

import sys

if "/opt/trn_rl_repo" not in sys.path:
    sys.path.insert(0, "/opt/trn_rl_repo")

import numpy as np

B, T, D, H = 8, 4096, 768, 768
TC = 512                    # steady-state time-chunk (one PSUM bank of fp32)
KD = D // 128               # 6 contraction tiles
MH = H // 128               # 6 hidden tiles
# (time offset, width, x-load index, offset within load)
CHUNKS = [(c * TC, TC, c, 0) for c in range(8)]
XLOADS = [(c * TC, TC) for c in range(8)]
EPILOG = 5   # h-groups of the last chunk folded into the last gate iteration

_state = {}


def _build():
    import concourse.mybir as mybir
    import concourse.tile as tile
    from concourse import bacc

    f32, bf16 = mybir.dt.float32, mybir.dt.bfloat16
    A = mybir.AluOpType
    Act = mybir.ActivationFunctionType

    nc = bacc.Bacc("TRN2", target_bir_lowering=False, debug=False, num_devices=B)

    xP = nc.dram_tensor("xP", [128, KD, T], bf16, kind="ExternalInput")
    w_d = {p: nc.dram_tensor(f"w{p}", [128, KD, H], bf16, kind="ExternalInput") for p in "fih"}
    bc_d = nc.dram_tensor("bcat", [128, 4 * MH], f32, kind="ExternalInput")
    hT = nc.dram_tensor("hT", [H, T], bf16, kind="ExternalOutput")

    with tile.TileContext(nc) as tc:
        with (
            tc.tile_pool(name="wpool", bufs=1) as wpool,
            tc.tile_pool(name="cpool", bufs=1) as cpool,
            tc.tile_pool(name="xpool", bufs=3) as xpool,
            tc.tile_pool(name="pspool", bufs=8, space="PSUM") as pspool,
            tc.tile_pool(name="wk", bufs=6) as wk,
            tc.tile_pool(name="hpool", bufs=3) as hpool,
        ):
            # Head DMAs: chunk-0 x per kd on sync; wf per kd on gpsimd;
            # biases, wi (per kd), wh (kd pairs) on scalar.
            x0k = []
            for kd in range(KD):
                t = xpool.tile([128, TC], bf16, tag=f"x0k{kd}", name=f"x0k{kd}")
                nc.sync.dma_start(t[:], xP[:, kd, 0:TC])
                x0k.append(t)
            w_sb = {p: [] for p in "fih"}
            for kd in range(KD):
                t = wpool.tile([128, 1, H], bf16, tag=f"wf{kd}", name=f"wf{kd}")
                nc.gpsimd.dma_start(t[:], w_d["f"][:, kd:kd + 1, :])
                w_sb["f"].append(t)
            bcat = cpool.tile([128, 4 * MH], f32, tag="bc")
            nc.scalar.dma_start(bcat[:], bc_d[:])
            for kd in range(KD):
                t = wpool.tile([128, 1, H], bf16, tag=f"wi{kd}", name=f"wi{kd}")
                nc.scalar.dma_start(t[:], w_d["i"][:, kd:kd + 1, :])
                w_sb["i"].append(t)
            for kp in range(KD // 2):
                t = wpool.tile([128, 2, H], bf16, tag=f"wh{kp}", name=f"wh{kp}")
                nc.scalar.dma_start(t[:], w_d["h"][:, 2 * kp:2 * kp + 2, :])
                w_sb["h"].append(t)
            b_sb = {"f": bcat[:, 0:MH], "i": bcat[:, MH:2 * MH], "h": bcat[:, 2 * MH:3 * MH]}
            h0_sb = bcat[:, 3 * MH:4 * MH]
            two_sb = cpool.tile([128, 1], f32, tag="two")
            nc.gpsimd.memset(two_sb[:], 2.0)

            def wslice(p, kd, j):
                if p == "h":
                    return w_sb["h"][kd // 2][:, kd % 2:kd % 2 + 1, j * 128:(j + 1) * 128]
                return w_sb[p][kd][:, :, j * 128:(j + 1) * 128]

            xtiles = [None] * len(XLOADS)

            def xslice(ci, kd):
                _, w, li, off = CHUNKS[ci]
                if li == 0:
                    return x0k[kd][:, 0:w]
                return xtiles[li][:, kd:kd + 1, off:off + w]

            def load_x(li):
                if li == 0 or xtiles[li] is not None:
                    return
                t0x, wx = XLOADS[li]
                xt = xpool.tile([128, KD, TC], bf16, tag="x", name=f"x{li}")
                nc.sync.dma_start(xt[:, :, 0:wx], xP[:, :, t0x:t0x + wx])
                xtiles[li] = xt

            PS = [[dict() for _ in range(MH)] for _ in CHUNKS]
            SG = [[None] * MH for _ in CHUNKS]
            prev_h = [None] * MH


            def emit_group(ci, p, j):
                w = CHUNKS[ci][1]
                pt = pspool.tile([128, TC], f32, tag="ps", name=f"ps{ci}_{j}_{p}")
                for kd in range(KD):
                    nc.tensor.matmul(
                        pt[:, 0:w],
                        wslice(p, kd, j),
                        xslice(ci, kd),
                        start=(kd == 0),
                        stop=(kd == KD - 1),
                    )
                PS[ci][j][p] = pt

            def stage1(ci, j):
                w = CHUNKS[ci][1]
                ps = PS[ci][j]
                ef = wk.tile([128, TC], bf16, tag="ef", name=f"ef{ci}_{j}")
                nc.scalar.activation(ef[:, 0:w], ps["f"][:, 0:w], Act.Exp, bias=b_sb["f"][:, j:j + 1], scale=-1.0)
                ei = wk.tile([128, TC], bf16, tag="ei", name=f"ei{ci}_{j}")
                nc.scalar.activation(ei[:, 0:w], ps["i"][:, 0:w], Act.Exp, bias=b_sb["i"][:, j:j + 1], scale=-1.0)
                ss = wk.tile([128, TC], bf16, tag="s2", name=f"ss{ci}_{j}")
                nc.vector.tensor_tensor(ss[:, 0:w], ef[:, 0:w], ei[:, 0:w], A.add)
                SG[ci][j] = [ei, ss, None, None]

            def stage2a(ci, j):
                w = CHUNKS[ci][1]
                ei, ss, _, _ = SG[ci][j]
                ln2 = wk.tile([128, TC], f32, tag="ln2", name=f"ln{ci}_{j}")
                nc.scalar.activation(ln2[:, 0:w], ss[:, 0:w], Act.Ln, bias=two_sb[:, 0:1], scale=1.0)
                rt = wk.tile([128, TC], bf16, tag="rt", name=f"rt{ci}_{j}")
                nc.scalar.activation(rt[:, 0:w], ln2[:, 0:w], Act.Exp, bias=0.0, scale=-1.0)
                at = wk.tile([128, TC], bf16, tag="a", name=f"at{ci}_{j}", bufs=14)
                nc.vector.scalar_tensor_tensor(at[:, 0:w], ei[:, 0:w], 1.0, rt[:, 0:w], A.add, A.mult)
                ut = wk.tile([128, TC], bf16, tag="u", name=f"ut{ci}_{j}", bufs=14)
                nc.vector.tensor_scalar(ut[:, 0:w], at[:, 0:w], scalar1=-1.0, scalar2=1.0, op0=A.mult, op1=A.add)
                SG[ci][j][2] = at
                SG[ci][j][3] = ut

            def stage2b(ci, j):
                tc0, w = CHUNKS[ci][0], CHUNKS[ci][1]
                ps = PS[ci][j]
                _, _, at, ut = SG[ci][j]
                bt = wk.tile([128, TC], bf16, tag="b", name=f"bt{ci}_{j}")
                nc.vector.scalar_tensor_tensor(bt[:, 0:w], ps["h"][:, 0:w], b_sb["h"][:, j:j + 1], ut[:, 0:w], A.add, A.mult)
                hh = hpool.tile([128, TC], bf16, tag=f"h{j}", name=f"hh{ci}_{j}")
                init = h0_sb[:, j:j + 1] if ci == 0 else prev_h[j][:, TC - 1:TC]
                nc.vector.tensor_tensor_scan(hh[:, 0:w], at[:, 0:w], bt[:, 0:w], init, op0=A.mult, op1=A.add)
                prev_h[j] = hh
                q = (nc.sync, nc.scalar, nc.gpsimd)[j % 3]
                q.dma_start(hT[j * 128:(j + 1) * 128, tc0:tc0 + w], hh[:, 0:w])

            # A(0): kd-outer in j-triples so the first matmuls need only
            # wf[kd0]+x0[kd0], and PSUM stays within 8 banks.
            load_x(1)
            for p in "fi":
                for jh in (range(0, 3), range(3, 6)):
                    for kd in range(KD):
                        for j in jh:
                            if kd == 0:
                                PS[0][j][p] = pspool.tile([128, TC], f32, tag="ps", name=f"ps0_{j}_{p}")
                            nc.tensor.matmul(
                                PS[0][j][p][:, 0:TC],
                                wslice(p, kd, j),
                                xslice(0, kd),
                                start=(kd == 0),
                                stop=(kd == KD - 1),
                            )
            for j in range(MH):
                stage1(0, j)
            for j in range(MH):
                stage2a(0, j)

            last = len(CHUNKS) - 1
            for ci in range(1, len(CHUNKS)):
                load_x(CHUNKS[min(ci + 1, last)][2])
                zero_lag = ci == last
                for j in range(MH):
                    emit_group(ci, "f", j)
                    emit_group(ci, "i", j)
                    stage1(ci, j)
                    if zero_lag:
                        # final chunk: no pipeline lag so its gate chains
                        # clear the engines before the closing h-phase
                        stage2a(ci, j)
                    emit_group(ci - 1, "h", j)
                    stage2b(ci - 1, j)
                    if zero_lag and j >= MH - EPILOG:
                        # fold the last chunk's first h-groups in here so
                        # only EPILOG scans trail the final matmul
                        jj = j - (MH - EPILOG)
                        emit_group(last, "h", jj)
                        stage2b(last, jj)
                    if not zero_lag and j > 0:
                        stage2a(ci, j - 1)
                if not zero_lag:
                    stage2a(ci, MH - 1)
            for j in range(EPILOG, MH):
                emit_group(last, "h", j)
                stage2b(last, j)

    # All our ACT funcs (Exp, Ln) live in the single table
    # "natural_log_exp_and_others"; empty every other table so the
    # table-load pass emits exactly one load (names/positions preserved).
    import concourse.bacc as bacc_mod

    orig_tables = bacc_mod.get_activation_tables

    def _single_table(arch):
        tabs = orig_tables(arch)
        keep = "natural_log_exp_and_others"
        return {k: (v if k == keep else set()) for k, v in tabs.items()}

    bacc_mod.get_activation_tables = _single_table
    try:
        nc.compile()
    finally:
        bacc_mod.get_activation_tables = orig_tables
    return nc


def _get_nc():
    if "nc" not in _state:
        _state["nc"] = _build()
    return _state["nc"]


def _prep_inputs(x, h0, f_w, f_b, i_w, i_b, h_w, h_b):
    import ml_dtypes

    bf16 = ml_dtypes.bfloat16
    x = np.asarray(x, dtype=np.float32)
    h0 = np.asarray(h0, dtype=np.float32)
    # [B, D, T] -> kd-major [B, 128, KD, T] so each chunk is one DMA
    xT = x.transpose(0, 2, 1).reshape(B, KD, 128, T).transpose(0, 2, 1, 3)
    xT = np.ascontiguousarray(xT.astype(bf16))
    shared = {}
    biases = []
    for p, wgt, bias, sgn in (("f", f_w, f_b, -1.0), ("i", i_w, i_b, -1.0), ("h", h_w, h_b, 1.0)):
        wgt = np.asarray(wgt, dtype=np.float32)
        # f/i biases negated: kernel computes Exp(-pre + bias_ap), needs bias_ap = -b
        biases.append((sgn * np.asarray(bias, dtype=np.float32)).reshape(MH, 128).T)
        wP = wgt.T.reshape(KD, 128, H).transpose(1, 0, 2)   # [128, KD, H]
        shared[f"w{p}"] = np.ascontiguousarray(wP.astype(bf16))
    in_maps = []
    for b in range(B):
        m = dict(shared)
        m["xP"] = xT[b]
        h0c = h0[b, 0].reshape(MH, 128).T
        m["bcat"] = np.ascontiguousarray(np.concatenate(biases + [h0c], axis=1))
        in_maps.append(m)
    return in_maps


def kernel(x, h0, f_w, f_b, i_w, i_b, h_w, h_b, _trace=False):
    from concourse.bass_utils import run_bass_kernel_spmd

    nc = _get_nc()
    in_maps = _prep_inputs(x, h0, f_w, f_b, i_w, i_b, h_w, h_b)
    res = run_bass_kernel_spmd(nc, in_maps, core_ids=list(range(B)), trace=_trace)
    out = np.empty((B, T, H), dtype=np.float32)
    for b in range(B):
        out[b] = res.results[b]["hT"].T.astype(np.float32)
    if _trace:
        _state["last_results"] = res
    return out


# revision 26
# speedup vs baseline: 1.2474x; 1.0125x over previous
"""MinLSTM cell for Trainium2 (Bass/Tile), data-parallel over batch on 8 cores.

Per core (one batch row), software-pipelined at j-granularity, one chunk of
lag between the gate side and the h side:

  step (ci, j):  f(ci,j), i(ci,j)   - bf16 matmuls, K=768 in PSUM
                 stage1(ci,j)       - ef, ei (ACT, from PSUM), ss (DVE)
                 h(ci-1,j)          - bf16 matmuls
                 stage2b(ci-1,j)    - bt (DVE), scan (DVE), store
                 stage2a(ci,j-1)    - ln, rt (ACT), at, ut (DVE)

All elementwise work sits on ACT+DVE; the Pool engine only issues the wf
load (its SBUF port is shared with DVE, so Pool compute slows DVE scans).

The last chunk runs its gate chain with zero pipeline lag and folds half of
its h-groups into the final gate iteration, so only EPILOG h/bt/scan/store
chains trail the final gate matmul.

Gates are division-free via Exp/Ln from one LUT table: Ef=e^{-zf},
Ei=e^{-zi}, a = (1+Ei)/(2+Ef+Ei) with 1/s2 = Exp(-Ln(ss+2)), u = 1-a.

DMA queues (3 usable, ~88GB/s each): sync = bf16 x + even-j stores,
scalar = biases, wi, wh, odd-j stores, gpsimd = wf (loads never block the
queue; stores wait on their scan, so they're split across two queues).
Chunk 0's x and wf/wi come as per-kd tiles so the first matmul only waits
for one kd slice. Output hT [H,T] bf16; host transposes/upcasts to fp32.
"""

import sys

if "/opt/trn_rl_repo" not in sys.path:
    sys.path.insert(0, "/opt/trn_rl_repo")

import numpy as np

B, T, D, H = 8, 4096, 768, 768
TC = 512                    # steady-state time-chunk (one PSUM bank of fp32)
KD = D // 128               # 6 contraction tiles
MH = H // 128               # 6 hidden tiles
# (time offset, width, x-load index, offset within load)
CHUNKS = [(c * TC, TC, c, 0) for c in range(8)]
XLOADS = [(c * TC, TC) for c in range(8)]
EPILOG = 5   # h-groups of the last chunk folded into the last gate iteration

_state = {}


def _build():
    import concourse.mybir as mybir
    import concourse.tile as tile
    from concourse import bacc

    f32, bf16 = mybir.dt.float32, mybir.dt.bfloat16
    A = mybir.AluOpType
    Act = mybir.ActivationFunctionType

    nc = bacc.Bacc("TRN2", target_bir_lowering=False, debug=False, num_devices=B)

    xP = nc.dram_tensor("xP", [128, KD, T], bf16, kind="ExternalInput")
    w_d = {p: nc.dram_tensor(f"w{p}", [128, KD, H], bf16, kind="ExternalInput") for p in "fih"}
    bc_d = nc.dram_tensor("bcat", [128, 4 * MH], f32, kind="ExternalInput")
    hT = nc.dram_tensor("hT", [H, T], bf16, kind="ExternalOutput")

    with tile.TileContext(nc) as tc:
        with (
            tc.tile_pool(name="wpool", bufs=1) as wpool,
            tc.tile_pool(name="cpool", bufs=1) as cpool,
            tc.tile_pool(name="xpool", bufs=3) as xpool,
            tc.tile_pool(name="pspool", bufs=8, space="PSUM") as pspool,
            tc.tile_pool(name="wk", bufs=6) as wk,
            tc.tile_pool(name="hpool", bufs=3) as hpool,
        ):
            # Head DMAs: the chunk-0 kd-outer f-phase consumes (x0k[kd],
            # wf[kd]) pairs every ~1.35us but one queue delivers a tile only
            # every ~2.2us — round-robin the pairs across all three queues in
            # consumption order so no single queue gates the stream start.
            bcat = cpool.tile([128, 4 * MH], f32, tag="bc")
            nc.scalar.dma_start(bcat[:], bc_d[:])
            QS = (nc.sync, nc.gpsimd, nc.scalar)
            x0k, w_sb = [], {p: [] for p in "fih"}
            for kd in range(KD):
                t = xpool.tile([128, TC], bf16, tag=f"x0k{kd}", name=f"x0k{kd}")
                QS[(2 * kd) % 3].dma_start(t[:], xP[:, kd, 0:TC])
                x0k.append(t)
                t = wpool.tile([128, 1, H], bf16, tag=f"wf{kd}", name=f"wf{kd}")
                QS[(2 * kd + 1) % 3].dma_start(t[:], w_d["f"][:, kd:kd + 1, :])
                w_sb["f"].append(t)
            for kd in range(KD):
                t = wpool.tile([128, 1, H], bf16, tag=f"wi{kd}", name=f"wi{kd}")
                QS[kd % 3].dma_start(t[:], w_d["i"][:, kd:kd + 1, :])
                w_sb["i"].append(t)
            for kp in range(KD // 2):
                t = wpool.tile([128, 2, H], bf16, tag=f"wh{kp}", name=f"wh{kp}")
                QS[kp % 3].dma_start(t[:], w_d["h"][:, 2 * kp:2 * kp + 2, :])
                w_sb["h"].append(t)
            b_sb = {"f": bcat[:, 0:MH], "i": bcat[:, MH:2 * MH], "h": bcat[:, 2 * MH:3 * MH]}
            h0_sb = bcat[:, 3 * MH:4 * MH]
            two_sb = cpool.tile([128, 1], f32, tag="two")
            nc.gpsimd.memset(two_sb[:], 2.0)

            def wslice(p, kd, j):
                if p == "h":
                    return w_sb["h"][kd // 2][:, kd % 2:kd % 2 + 1, j * 128:(j + 1) * 128]
                return w_sb[p][kd][:, :, j * 128:(j + 1) * 128]

            xtiles = [None] * len(XLOADS)

            def xslice(ci, kd):
                _, w, li, off = CHUNKS[ci]
                if li == 0:
                    return x0k[kd][:, 0:w]
                return xtiles[li][:, kd:kd + 1, off:off + w]

            def load_x(li):
                if li == 0 or xtiles[li] is not None:
                    return
                t0x, wx = XLOADS[li]
                xt = xpool.tile([128, KD, TC], bf16, tag="x", name=f"x{li}")
                nc.sync.dma_start(xt[:, :, 0:wx], xP[:, :, t0x:t0x + wx])
                xtiles[li] = xt

            PS = [[dict() for _ in range(MH)] for _ in CHUNKS]
            SG = [[None] * MH for _ in CHUNKS]
            prev_h = [None] * MH


            def emit_group(ci, p, j):
                w = CHUNKS[ci][1]
                pt = pspool.tile([128, TC], f32, tag="ps", name=f"ps{ci}_{j}_{p}")
                for kd in range(KD):
                    nc.tensor.matmul(
                        pt[:, 0:w],
                        wslice(p, kd, j),
                        xslice(ci, kd),
                        start=(kd == 0),
                        stop=(kd == KD - 1),
                    )
                PS[ci][j][p] = pt

            def stage1(ci, j):
                w = CHUNKS[ci][1]
                ps = PS[ci][j]
                ef = wk.tile([128, TC], bf16, tag="ef", name=f"ef{ci}_{j}")
                nc.scalar.activation(ef[:, 0:w], ps["f"][:, 0:w], Act.Exp, bias=b_sb["f"][:, j:j + 1], scale=-1.0)
                ei = wk.tile([128, TC], bf16, tag="ei", name=f"ei{ci}_{j}")
                nc.scalar.activation(ei[:, 0:w], ps["i"][:, 0:w], Act.Exp, bias=b_sb["i"][:, j:j + 1], scale=-1.0)
                ss = wk.tile([128, TC], bf16, tag="s2", name=f"ss{ci}_{j}")
                nc.vector.tensor_tensor(ss[:, 0:w], ef[:, 0:w], ei[:, 0:w], A.add)
                SG[ci][j] = [ei, ss, None, None]

            def stage2a(ci, j):
                w = CHUNKS[ci][1]
                ei, ss, _, _ = SG[ci][j]
                ln2 = wk.tile([128, TC], f32, tag="ln2", name=f"ln{ci}_{j}")
                nc.scalar.activation(ln2[:, 0:w], ss[:, 0:w], Act.Ln, bias=two_sb[:, 0:1], scale=1.0)
                rt = wk.tile([128, TC], bf16, tag="rt", name=f"rt{ci}_{j}")
                nc.scalar.activation(rt[:, 0:w], ln2[:, 0:w], Act.Exp, bias=0.0, scale=-1.0)
                at = wk.tile([128, TC], bf16, tag="a", name=f"at{ci}_{j}", bufs=10)
                nc.vector.scalar_tensor_tensor(at[:, 0:w], ei[:, 0:w], 1.0, rt[:, 0:w], A.add, A.mult)
                ut = wk.tile([128, TC], bf16, tag="u", name=f"ut{ci}_{j}", bufs=10)
                nc.vector.tensor_scalar(ut[:, 0:w], at[:, 0:w], scalar1=-1.0, scalar2=1.0, op0=A.mult, op1=A.add)
                SG[ci][j][2] = at
                SG[ci][j][3] = ut

            def stage2b(ci, j):
                tc0, w = CHUNKS[ci][0], CHUNKS[ci][1]
                ps = PS[ci][j]
                _, _, at, ut = SG[ci][j]
                bt = wk.tile([128, TC], bf16, tag="b", name=f"bt{ci}_{j}")
                nc.vector.scalar_tensor_tensor(bt[:, 0:w], ps["h"][:, 0:w], b_sb["h"][:, j:j + 1], ut[:, 0:w], A.add, A.mult)
                hh = hpool.tile([128, TC], bf16, tag=f"h{j}", name=f"hh{ci}_{j}")
                init = h0_sb[:, j:j + 1] if ci == 0 else prev_h[j][:, TC - 1:TC]
                nc.vector.tensor_tensor_scan(hh[:, 0:w], at[:, 0:w], bt[:, 0:w], init, op0=A.mult, op1=A.add)
                prev_h[j] = hh
                q = (nc.sync, nc.scalar, nc.gpsimd)[j % 3]
                q.dma_start(hT[j * 128:(j + 1) * 128, tc0:tc0 + w], hh[:, 0:w])

            # A(0): kd-outer in j-triples so the first matmuls need only
            # wf[kd0]+x0[kd0], and PSUM stays within 8 banks.
            load_x(1)
            for p in "fi":
                for jh in (range(0, 3), range(3, 6)):
                    for kd in range(KD):
                        for j in jh:
                            if kd == 0:
                                PS[0][j][p] = pspool.tile([128, TC], f32, tag="ps", name=f"ps0_{j}_{p}")
                            nc.tensor.matmul(
                                PS[0][j][p][:, 0:TC],
                                wslice(p, kd, j),
                                xslice(0, kd),
                                start=(kd == 0),
                                stop=(kd == KD - 1),
                            )
            for j in range(MH):
                stage1(0, j)
            for j in range(MH):
                stage2a(0, j)

            last = len(CHUNKS) - 1
            for ci in range(1, len(CHUNKS)):
                load_x(CHUNKS[min(ci + 1, last)][2])
                zero_lag = ci == last
                for j in range(MH):
                    emit_group(ci, "f", j)
                    emit_group(ci, "i", j)
                    stage1(ci, j)
                    if zero_lag:
                        # final chunk: no pipeline lag so its gate chains
                        # clear the engines before the closing h-phase
                        stage2a(ci, j)
                    emit_group(ci - 1, "h", j)
                    stage2b(ci - 1, j)
                    if zero_lag and j >= MH - EPILOG:
                        # fold the last chunk's first h-groups in here so
                        # only EPILOG scans trail the final matmul
                        jj = j - (MH - EPILOG)
                        emit_group(last, "h", jj)
                        stage2b(last, jj)
                    if not zero_lag and j > 0:
                        stage2a(ci, j - 1)
                if not zero_lag:
                    stage2a(ci, MH - 1)
            for j in range(EPILOG, MH):
                emit_group(last, "h", j)
                stage2b(last, j)

    # All our ACT funcs (Exp, Ln) live in the single table
    # "natural_log_exp_and_others"; empty every other table so the
    # table-load pass emits exactly one load (names/positions preserved).
    import concourse.bacc as bacc_mod

    orig_tables = bacc_mod.get_activation_tables

    def _single_table(arch):
        tabs = orig_tables(arch)
        keep = "natural_log_exp_and_others"
        return {k: (v if k == keep else set()) for k, v in tabs.items()}

    bacc_mod.get_activation_tables = _single_table
    try:
        nc.compile()
    finally:
        bacc_mod.get_activation_tables = orig_tables
    return nc


def _get_nc():
    if "nc" not in _state:
        _state["nc"] = _build()
    return _state["nc"]


def _prep_inputs(x, h0, f_w, f_b, i_w, i_b, h_w, h_b):
    import ml_dtypes

    bf16 = ml_dtypes.bfloat16
    x = np.asarray(x, dtype=np.float32)
    h0 = np.asarray(h0, dtype=np.float32)
    # [B, D, T] -> kd-major [B, 128, KD, T] so each chunk is one DMA
    xT = x.transpose(0, 2, 1).reshape(B, KD, 128, T).transpose(0, 2, 1, 3)
    xT = np.ascontiguousarray(xT.astype(bf16))
    shared = {}
    biases = []
    for p, wgt, bias, sgn in (("f", f_w, f_b, -1.0), ("i", i_w, i_b, -1.0), ("h", h_w, h_b, 1.0)):
        wgt = np.asarray(wgt, dtype=np.float32)
        # f/i biases negated: kernel computes Exp(-pre + bias_ap), needs bias_ap = -b
        biases.append((sgn * np.asarray(bias, dtype=np.float32)).reshape(MH, 128).T)
        wP = wgt.T.reshape(KD, 128, H).transpose(1, 0, 2)   # [128, KD, H]
        shared[f"w{p}"] = np.ascontiguousarray(wP.astype(bf16))
    in_maps = []
    for b in range(B):
        m = dict(shared)
        m["xP"] = xT[b]
        h0c = h0[b, 0].reshape(MH, 128).T
        m["bcat"] = np.ascontiguousarray(np.concatenate(biases + [h0c], axis=1))
        in_maps.append(m)
    return in_maps


def kernel(x, h0, f_w, f_b, i_w, i_b, h_w, h_b, _trace=False):
    from concourse.bass_utils import run_bass_kernel_spmd

    nc = _get_nc()
    in_maps = _prep_inputs(x, h0, f_w, f_b, i_w, i_b, h_w, h_b)
    res = run_bass_kernel_spmd(nc, in_maps, core_ids=list(range(B)), trace=_trace)
    out = np.empty((B, T, H), dtype=np.float32)
    for b in range(B):
        out[b] = res.results[b]["hT"].T.astype(np.float32)
    if _trace:
        _state["last_results"] = res
    return out
